# revision 30
# baseline (speedup 1.0000x reference)
"""Bass/Trainium2 kernel for nn_Expert_WNO2d (8-expert gated WaveConv2d mixture).

Math: the reference is linear in x. Every expert passes the fine Haar detail
levels (1..3) through unchanged and only channel-mixes the coarsest (level-4)
approximation + detail coefficients. With gate slots s weighting experts
PERM = (0,1,2,3,4,5,4,5), the output collapses to

    y[b] = G[b] * x[b] + rep8( adj[b] )                      (rep8 = 8x8 block broadcast)
    adj[b] = idwt4( sum_e geff[b,e] * (W_e . c4[b]) )*0.0625 - (G[b]/64) * s8[b]

where s8 = 8x8 block sums of x, c4 = level-4 Haar coefficients (from s8),
G[b] = sum_s lambda[b,s], geff[b,e] = gate mass routed to expert e.

Sharding: data-parallel over batch B=32 across 8 cores (4 samples/core).

I/O precision (tolerance is rel 2e-2; this lands ~7e-3): x and y travel as
bf16 (host cast), expert weights as fp8 e4m3 scaled by 2^12 into fp8's
normal range; ALL descale factors are folded into the host-packed gates.

Schedule: all HWDGE DMA data drains through one FIFO ring, so every DMA
issues from the sync engine in consumption order (x-rt0, gates, W bands,
x-rt1, y stores). 8x8 block sums use a dense bf16 pairwise tree over h
(2x DVE) + per-chunk w-direction tensor_reduce. Matmuls pack mode pairs
into 128-col fp8 lhsT (FWL) with N split by row-tile and a PSUM tile per
row-tile so rt0's synthesis never waits on rt1's matmuls. Junk matmuls
keyed on intermediate tiles keep the PE HAM clock at 8/8 across the span.
The output pass splits per row-tile across engine paths: chunk 0 goes
diag(G)@x + per-w-offset identity@adj accumulated in PSUM on the PE, then
drained to bf16 by the scalar engine; chunk 1 is a DVE broadcast-add onto
the scalar-engine-precomputed G*x. GPSIMD takes half of each cc build.
"""

import numpy as np

import concourse.bacc as bacc
import concourse.mybir as mybir
import concourse.tile as tile

N_CORES = 8
B, C, S = 32, 64, 64
BL = B // N_CORES          # samples per core = 4
NE = 6                     # live experts
NCH = 2                    # x chunks per row-tile, [128, 2048] each
f32 = mybir.dt.float32
bf16 = mybir.dt.bfloat16
fp16 = mybir.dt.float16
fp8 = mybir.dt.float8e4
ALU = mybir.AluOpType
AX = mybir.AxisListType
AF = mybir.ActivationFunctionType

W_SCALE = 4096.0           # host weight scale into fp8 normal range
GATE_DESCALE = 1.0 / (W_SCALE * 16.0 * 16.0 * 4.0)  # 2^-20: fp8 descale + 0.0625 fold


def _build_nc():
    nc = bacc.Bacc()
    xw = nc.declare_dram_parameter("xw", [2, 128, 4096], bf16, isOutput=False)
    wt = nc.declare_dram_parameter("wt", [4, 128, 3072], fp8, isOutput=False)
    gt = nc.declare_dram_parameter("gt", [128, 16], f32, isOutput=False)
    idg = nc.declare_dram_parameter("idg", [2, 128, 128], fp16, isOutput=False)
    idm = nc.declare_dram_parameter("idm", [128, 128], fp16, isOutput=False)
    yw = nc.declare_dram_parameter("yw", [2, 128, 4096], bf16, isOutput=True)

    with tile.TileContext(nc) as tc:
        with (
            tc.tile_pool(name="xp", bufs=4) as xp,
            tc.tile_pool(name="yp", bufs=1) as yp,
            tc.tile_pool(name="wp", bufs=4) as wp,
            tc.tile_pool(name="sp", bufs=2) as sp,
            tc.tile_pool(name="tp", bufs=4) as ttp,
            tc.tile_pool(name="ps", bufs=2, space="PSUM") as psp,
            tc.tile_pool(name="psy", bufs=2, space="PSUM") as psy,
        ):
            # per-rt PSUM tile, cols = band*32 + mp*4 + j*2 + b2; cols 128+
            # are the junk-matmul keepalive target
            pq = [psp.tile([128, 136], f32, tag="pq", name=f"pq{rt}") for rt in range(2)]
            yps = [psy.tile([128, 1024], f32, tag="yps", name=f"yps{h}") for h in range(2)]
            pjk0 = pq[0][0:32, 128:129]

            # ---- PE warmup: junk matmuls with no data deps push HAM to 8/8
            junk = sp.tile([128, 32], bf16, tag="junk", name="junk")
            nc.gpsimd.memset(junk[:, :], 0.0)
            for i in range(72):
                nc.tensor.matmul(
                    out=pjk0, lhsT=junk[:, 0:32], rhs=junk[:, 0:1],
                    start=True, stop=True,
                )

            def pe_keepalive(dep_ap, n=3):
                # junk matmuls whose rhs touches a just-produced tile: they
                # execute right after it, keeping the PE HAM window busy
                for i in range(n):
                    nc.tensor.matmul(
                        out=pjk0, lhsT=junk[:, 0:32], rhs=dep_ap,
                        start=True, stop=True,
                    )

            # ---- DMA in: single FIFO ring -> issue in consumption order
            xs = [[], []]
            wt_b = [wp.tile([128, 3072], fp8, tag="wt", name=f"w{band}")
                    for band in range(4)]
            gt_s = sp.tile([128, 16], f32, tag="gt", name="gt")
            idg_s = [sp.tile([128, 128], fp16, tag="idg", name=f"idg{rt}")
                     for rt in range(2)]
            idm_s = sp.tile([128, 128], fp16, tag="idm", name="idm")

            def load_x(rt):
                for c in range(NCH):
                    xt = xp.tile([128, 2048], bf16, tag="xs", name=f"x{rt}{c}")
                    nc.sync.dma_start(out=xt[:, :], in_=xw[rt, :, 2048 * c:2048 * (c + 1)])
                    xs[rt].append(xt)

            load_x(0)
            nc.sync.dma_start(out=gt_s[:, :], in_=gt[:, :])
            load_x(1)
            for band in range(4):
                nc.sync.dma_start(out=wt_b[band][:, :], in_=wt[band, :, :])
            for rt in range(2):
                nc.sync.dma_start(out=idg_s[rt][:, :], in_=idg[rt, :, :])
            nc.sync.dma_start(out=idm_s[:, :], in_=idm[:, :])

            # ---- G*x for the DVE-path chunk (c1) on the scalar engine
            ys_g = [None, None]
            with tc.tile_wait_until(1):
                for rt in range(2):
                    yg = yp.tile([128, 2048], bf16, tag=f"ysg{rt}", name=f"yg{rt}")
                    nc.scalar.activation(
                        out=yg[:, :], in_=xs[rt][1][:, :], func=AF.Copy,
                        scale=gt_s[:, 8 * rt:8 * rt + 1],
                    )
                    ys_g[rt] = yg

            # ---- per-rt coefficient chain ------------------------------
            # chunk cols = (h=32 -> 4 h-blocks, v=8, w=8). Dense bf16
            # pairwise tree over h within each 8-row block, then a per-chunk
            # w-direction tensor_reduce into this chunk's s8 half.
            s8 = []
            cc = sp.tile([128, 768], bf16, tag="cc", name="cc")

            def tree_chunk(rt, c, s8t):
                a = xs[rt][c]
                t1 = ttp.tile([128, 1024], bf16, tag="t1", name=f"t1{rt}{c}")
                v = lambda t, n: t[:, :].rearrange("p (hb q) -> p hb q", hb=4, q=n)
                av = a[:, :].rearrange("p (hb h2 q) -> p hb h2 q", hb=4, h2=2, q=256)
                nc.vector.tensor_add(v(t1, 256), av[:, :, 0], av[:, :, 1])
                t2 = ttp.tile([128, 512], bf16, tag="t2", name=f"t2{rt}{c}")
                t1v = t1[:, :].rearrange("p (hb h2 q) -> p hb h2 q", hb=4, h2=2, q=128)
                nc.vector.tensor_add(v(t2, 128), t1v[:, :, 0], t1v[:, :, 1])
                h3 = ttp.tile([128, 256], bf16, tag="h3", name=f"h3{rt}{c}")
                t2v = t2[:, :].rearrange("p (hb h2 q) -> p hb h2 q", hb=4, h2=2, q=64)
                nc.vector.tensor_add(v(h3, 64), t2v[:, :, 0], t2v[:, :, 1])
                # w-reduce into s8 cols for this chunk's 4 h-blocks
                nc.vector.tensor_reduce(
                    out=s8t[:, 32 * c:32 * (c + 1)].rearrange("p (u v) -> p u v", u=4),
                    in_=h3[:, :].rearrange("p (u v w) -> p u v w", u=4, v=8, w=8),
                    axis=AX.X, op=ALU.add,
                )
                return h3

            def analysis(rt):
                s8t = s8[rt]
                # level-4 Haar analysis directly on s8 (scales live in gates)
                ev = s8t[:, 0:64].rearrange("p (x i y j) -> p i j x y",
                                            x=4, i=2, y=4, j=2)[:, :, 0]
                od = s8t[:, 0:64].rearrange("p (x i y j) -> p i j x y",
                                            x=4, i=2, y=4, j=2)[:, :, 1]
                tt = ttp.tile([128, 64], f32, tag="tt", name=f"tt{rt}")
                t2v2 = lambda o: tt[:, 32 * o:32 * (o + 1)].rearrange(
                    "p (g x y) -> p g x y", g=2, x=4, y=4)
                nc.vector.tensor_add(t2v2(0), ev, od)
                nc.vector.tensor_sub(t2v2(1), ev, od)
                cf = sp.tile([128, 64], f32, tag="coef", name=f"cf{rt}")
                pick = lambda t, o: t[:, :].rearrange(
                    "p (g h m) -> p h g m", g=2, h=2, m=16)[:, o]
                nc.vector.tensor_add(pick(cf, 0), pick(tt, 0), pick(tt, 1))
                nc.vector.tensor_sub(pick(cf, 1), pick(tt, 0), pick(tt, 1))

                # gate-scaled coefficients, cc[el*64+i, ch*256 + q*4 + b]
                # (q = band*16+mode); half on DVE, half on GPSIMD
                ccv = cc[:, :].rearrange("p (ch q b) -> p b ch q", ch=3, q=64, b=4)
                for bh in range(2):
                    b = rt * 2 + bh
                    for el in range(2):
                        eng = nc.gpsimd if el == 1 else nc.vector
                        eng.tensor_tensor(
                            out=ccv[el * 64:(el + 1) * 64, b],
                            in0=cf[bh * 64:(bh + 1) * 64, :]
                                .rearrange("p (o q) -> p o q", o=1)
                                .broadcast_to([64, 3, 64]),
                            in1=gt_s[bh * 64:(bh + 1) * 64, 8 * rt + 1 + el:8 * rt + 6 + el:2]
                                .rearrange("p (c o) -> p c o", c=3, o=1)
                                .broadcast_to([64, 3, 64]),
                            op=ALU.mult,
                        )
                return cf

            def coeff_chain(rt):
                s8t = sp.tile([128, 64], f32, tag="s8", name=f"s8{rt}")
                s8.append(s8t)
                for c in range(NCH):
                    h3 = tree_chunk(rt, c, s8t)
                    pe_keepalive(h3[0:128, 0:1])
                return analysis(rt)

            # ---- matmuls: lhsT[128=(el,i), 128=(j,o)] per (band, mp, ch),
            # FWL-eligible fp8; rhs N=4 = (j=2, b=2) per rt into this rt's
            # own PSUM tile. K-accum over ch (expert pairs).
            def mm_block(rt):
                pbv = pq[rt][:, 0:128].rearrange("p (band mp j b) -> p band mp j b",
                                             band=4, mp=8, j=2, b=2)
                ccv = cc[:, :].rearrange("p (ch bm j b) -> p ch bm j b",
                                         ch=3, bm=32, j=2, b=4)
                for band in range(4):
                    for mp in range(8):
                        for ch in range(3):
                            nc.tensor.matmul(
                                out=pbv[:, band, mp],
                                lhsT=wt_b[band][:, (mp * 3 + ch) * 128:(mp * 3 + ch + 1) * 128],
                                rhs=ccv[:, ch, band * 8 + mp, :, 2 * rt:2 * rt + 2],
                                start=(ch == 0), stop=(ch == 2),
                            )

            # ---- synthesis + fused output pass --------------------------
            def synth(rt):
                # one PSUM->SBUF copy (walrus: DVE reads at most one PSUM
                # operand), then u13 = pb0 +/- pb1, u24 = pb2 +/- pb3 with
                # cols k*64 + m*4 + b2; valid pq quadrants are rows j*64+o
                # at cols mp*4 + j*2 + b2.
                pbs = sp.tile([128, 128], f32, tag="pbs", name=f"pbs{rt}")
                nc.vector.tensor_copy(out=pbs[:, :], in_=pq[rt][:, 0:128])
                u13 = ttp.tile([64, 64], f32, tag="u13", name=f"u13{rt}")
                u24 = ttp.tile([64, 64], f32, tag="u24", name=f"u24{rt}")
                for (u, lo, hi) in ((u13, 0, 1), (u24, 2, 3)):
                    uv = u[:, :].rearrange("p (k mp j b) -> p k j mp b",
                                           k=2, mp=8, j=2, b=2)
                    for j in range(2):
                        pv = lambda band: pbs[j * 64:(j + 1) * 64, :].rearrange(
                            "p (bd mp j2 b) -> p bd j2 mp b", bd=4, mp=8, j2=2, b=2)[:, band, j]
                        nc.vector.tensor_add(uv[:, 0, j], pv(lo), pv(hi))
                        nc.vector.tensor_sub(uv[:, 1, j], pv(lo), pv(hi))
                # idwt level-4 scatter + pass-through correction
                at = sp.tile([128, 64], f32, tag="adjT", name=f"at{rt}")
                for bh in range(2):
                    ov = at[bh * 64:(bh + 1) * 64, :].rearrange(
                        "p (x di y dj) -> p dj di x y", x=4, di=2, y=4, dj=2)
                    sv = lambda t: t[:, :].rearrange(
                        "p (k x y bb) -> p bb k x y", k=2, x=4, y=4, bb=2)[:, bh]
                    nc.vector.tensor_add(ov[:, 0], sv(u13), sv(u24))
                    nc.vector.tensor_sub(ov[:, 1], sv(u13), sv(u24))
                adjF = sp.tile([128, 64], f32, tag="adjF", name=f"af{rt}")
                nc.vector.scalar_tensor_tensor(
                    out=adjF[:, :], in0=s8[rt][:, :], scalar=gt_s[:, 8 * rt + 7:8 * rt + 8],
                    in1=at[:, :], op0=ALU.mult, op1=ALU.add,
                )
                adj_h = sp.tile([128, 512], bf16, tag="adjh", name=f"ah{rt}")
                nc.vector.tensor_copy(
                    out=adj_h[:, :].rearrange("p (u dh v) -> p u dh v", u=8, dh=8, v=8),
                    in_=adjF[:, :].rearrange("p (u o v) -> p u o v", u=8, o=1, v=8)
                        .broadcast_to([128, 8, 8, 8]),
                )
                return adj_h

            def y_pass(rt, adj_h):
                # chunk 0 via PE: yps = diag(G) @ x + I @ rep8(adj) accumulated
                # in PSUM per 1024-col half, drained by the scalar engine;
                # chunk 1 via DVE broadcast-add on the ACT-precomputed G*x.
                for k in range(2):
                    # same-lhsT matmuls adjacent: both diag(G)@x quarters,
                    # then both rep8(adj) quarters (one weight load each)
                    for q in range(2):
                        nc.tensor.matmul(
                            out=yps[k][:, 512 * q:512 * (q + 1)], lhsT=idg_s[rt][:, :],
                            rhs=xs[rt][0][:, 1024 * k + 512 * q:1024 * k + 512 * (q + 1)],
                            start=True, stop=False,
                        )
                    for q in range(2):
                        # rep8 over w in one matmul: out cols (w, hv), rhs
                        # broadcasts the 64 adj cols over the w dim
                        nc.tensor.matmul(
                            out=yps[k][:, 512 * q:512 * (q + 1)]
                                .rearrange("p (hv w) -> p w hv", w=8),
                            lhsT=idm_s[:, :],
                            rhs=adj_h[:, 128 * k + 64 * q:128 * k + 64 * (q + 1)]
                                .rearrange("p (o q2) -> p o q2", o=1)
                                .broadcast_to([128, 8, 64]),
                            start=False, stop=True, skip_group_check=True,
                        )
                    ysk = yp.tile([128, 1024], bf16, tag=f"ys{rt}0{k}", name=f"y{rt}0{k}")
                    nc.scalar.activation(out=ysk[:, :], in_=yps[k][:, :], func=AF.Copy)
                    nc.sync.dma_start(out=yw[rt, :, 1024 * k:1024 * (k + 1)],
                                      in_=ysk[:, :])
                ys1 = yp.tile([128, 2048], bf16, tag=f"ys{rt}1", name=f"y{rt}1")
                nc.vector.tensor_tensor(
                    out=ys1[:, :].rearrange("p (hv w) -> p hv w", w=8),
                    in0=ys_g[rt][:, :].rearrange("p (hv w) -> p hv w", w=8),
                    in1=adj_h[:, 256:512].rearrange("p (hv o) -> p hv o", o=1)
                        .broadcast_to([128, 256, 8]),
                    op=ALU.add,
                )
                nc.sync.dma_start(out=yw[rt, :, 2048:4096], in_=ys1[:, :])

            # explicit logical stages pin the per-engine order (the default
            # scheduler's DMA model doesn't know the FIFO ring sharing and
            # otherwise head-of-line-blocks DVE on MM-dependent synthesis)
            with tc.tile_wait_until(1):
                s8t0 = sp.tile([128, 64], f32, tag="s8", name="s80")
                s8.append(s8t0)
                h3 = tree_chunk(0, 0, s8t0)
                pe_keepalive(xs[0][0][0:128, 0:1])
                pe_keepalive(h3[0:128, 0:1])
            with tc.tile_wait_until(2):
                h3 = tree_chunk(0, 1, s8t0)
                pe_keepalive(h3[0:128, 0:1])
            with tc.tile_wait_until(3):
                s8t1 = sp.tile([128, 64], f32, tag="s8", name="s81")
                s8.append(s8t1)
                h3 = tree_chunk(1, 0, s8t1)
                pe_keepalive(h3[0:128, 0:1])
            with tc.tile_wait_until(4):
                analysis(0)
            with tc.tile_wait_until(5):
                pe_keepalive(cc[0:128, 0:1])
            with tc.tile_wait_until(6):
                mm_block(0)
            with tc.tile_wait_until(7):
                h3 = tree_chunk(1, 1, s8t1)
                pe_keepalive(h3[0:128, 0:1])
            with tc.tile_wait_until(8):
                analysis(1)
            with tc.tile_wait_until(9):
                pe_keepalive(cc[0:128, 2:3])
            with tc.tile_wait_until(10):
                mm_block(1)
            with tc.tile_wait_until(11):
                adj0 = synth(0)
            with tc.tile_wait_until(12):
                adj1 = synth(1)
            with tc.tile_wait_until(13):
                y_pass(0, adj0)
            with tc.tile_wait_until(14):
                y_pass(1, adj1)
    nc.compile()
    return nc


_NC = None


def _get_nc():
    global _NC
    if _NC is None:
        _NC = _build_nc()
    return _NC


def _pack_weights(WL, WH):
    import ml_dtypes
    # Wall[band, e, i, o, x, y]; band 0 = WL, bands 1..3 = WH[:, k-1]
    Wall = np.empty((4, NE, C, C, 4, 4), np.float32)
    Wall[0] = WL[:NE]
    for k in range(3):
        Wall[k + 1] = WH[:NE, k]
    Wall *= W_SCALE
    # wt[band][el*64+i, mp*384 + ch*128 + j*64 + o], e = ch*2+el, m = 2*mp+j
    W7 = Wall.reshape(4, 3, 2, C, C, 4, 4)            # band, ch, el, i, o, x, y
    T = W7.transpose(0, 2, 3, 5, 6, 1, 4)             # band, el, i, x, y, ch, o
    T = T.reshape(4, 2, C, 8, 2, 3, C)                # band, el, i, mp, j, ch, o
    T = T.transpose(0, 1, 2, 3, 5, 4, 6)              # band, el, i, mp, ch, j, o
    return np.ascontiguousarray(T.reshape(4, 128, 3072)).astype(ml_dtypes.float8_e4m3)


def _pack_gates(lambda_):
    lam = lambda_.reshape(B, 8).astype(np.float32)
    G = lam.sum(1)
    geff = lam[:, :6].copy()
    geff[:, 4] += lam[:, 6]
    geff[:, 5] += lam[:, 7]
    gtv = np.zeros((B, 8), np.float32)
    gtv[:, 0] = G
    gtv[:, 1:7] = geff * GATE_DESCALE
    gtv[:, 7] = -G / 64.0
    return gtv


def _build_in_maps(x, lambda_, WL, WH):
    import ml_dtypes
    wtp = _pack_weights(np.asarray(WL, np.float32), np.asarray(WH, np.float32))
    gtv = _pack_gates(np.asarray(lambda_, np.float32))
    xb = np.asarray(x, np.float32).astype(ml_dtypes.bfloat16)
    idm = np.eye(128, dtype=np.float16)

    in_maps = []
    for k in range(N_CORES):
        xl = xb[k * BL:(k + 1) * BL].reshape(2, 128, 4096)
        # gt[bh*64+i, rt*8 + col] = gtv[k*BL + rt*2 + bh, col]
        gl = np.empty((128, 16), np.float32)
        idgl = np.zeros((2, 128, 128), np.float16)
        for rt in range(2):
            for bh in range(2):
                b = k * BL + rt * 2 + bh
                gl[bh * 64:(bh + 1) * 64, rt * 8:(rt + 1) * 8] = gtv[b]
                sl = slice(bh * 64, (bh + 1) * 64)
                idgl[rt][sl, sl] = np.eye(64, dtype=np.float16) * np.float16(gtv[b, 0])
        in_maps.append({"xw": np.ascontiguousarray(xl),
                        "wt": wtp,
                        "gt": gl,
                        "idg": idgl,
                        "idm": idm})
    return in_maps


def kernel(x, lambda_, WL, WH):
    from concourse.bass_utils import run_bass_kernel_spmd

    nc = _get_nc()
    in_maps = _build_in_maps(x, lambda_, WL, WH)
    res = run_bass_kernel_spmd(nc, in_maps, list(range(N_CORES)))
    out = np.empty((B, C, S, S), np.float32)
    for k in range(N_CORES):
        out[k * BL:(k + 1) * BL] = np.asarray(
            res.results[k]["yw"], dtype=np.float32).reshape(BL, C, S, S)
    return out
